# revision 39
# baseline (speedup 1.0000x reference)
"""Per-donor routed linear layer on 8 Trainium2 cores.

out[i] = x[i] @ W[donor_labels[i]].T + b[donor_labels[i]]

Strategy: route on host (stable sort of rows by donor label), one donor per
core, zero-padded to a common row count R.  Each core runs a dense
(R x 1024) @ (1024 x 100) matmul with K tiled 8x128, accumulating in PSUM,
bias added on the DVE during PSUM->SBUF eviction.

The kernel is memory-bound on x, so x is quantized host-side to fp8 e3m4
(float8e3, 4 mantissa bits) with a 2x pre-scale folded into W — the TRN2 PE
accepts mixed-dtype matmuls (stationary W in fp16, moving x in fp8e3, probed
exact to fp32 rounding), so W contributes no quantization error and the PE
still runs at 1 cycle/row.  Measured rel err vs the fp32 reference: 1.30e-2
(deterministic), against a 2e-2 gate.  fp8e4+DoubleRow (0.5 cyc/row) fails
the gate (x in e4m3 alone measures 2.55e-2), and the PE has no 8-bit int
mode, so 1 byte/element at 1 cyc/row is the floor for both traffic and PE.

With x at 1B the stream is balanced at the edge: per core ~16.9MB of x +
~3.3MB of fp16 output vs the 16-engine DMA pool's ~340GB/s, and 33*8
matmuls of 512 rows = ~58us on the PE — both ~58-60us.  Layout and queue
choices keep them overlapped:
- x is pre-permuted on host into (n_pairs, 128 partitions, 8 k-tiles, 1024
  rows) so each partition's read per block-pair is one contiguous 8KB run
  (descriptors are per partition-run; big runs keep the pool at line rate).
- each pair is issued half on the sync queue, half on the scalar queue (the
  only two HWDGE engines) so the k-ordered matmul chain starts on the first
  half while the second streams; the first pair is further split into 2-k-
  tile chunks because the DMA engine pool comes up staggered over ~10us.
- output (fp16, gene-major (100, R), transposed/unpermuted on host) is
  stored in 8-block groups on the gpsimd (SWDGE) queue to stay out of x's
  way, then 2-block groups on sync/scalar at the tail so the post-matmul
  drain is only the last 2 blocks.
Measured: ~77-80us NEFF exec (from 126us for the fp16 predecessor); PE-dense
window ~58us with <1us of gaps, ~11-13us head (DMA pool bring-up, engine
staggered ~650ns each), ~6us drain+teardown.
"""

import sys

sys.path.insert(0, "/opt/trn_rl_repo")

import numpy as np
import ml_dtypes

import concourse.bacc as bacc
import concourse.mybir as mybir
from concourse.tile import TileContext
from concourse.bass_utils import run_bass_kernel_spmd

N_CORES = 8
N_DONORS = 8
D_IN = 1024
N_GENES = 100
K_TILES = D_IN // 128
BLOCK = 512  # moving rows per matmul (one fp32 PSUM bank)
PAIR = 2 * BLOCK  # rows per x DMA (8KB fp8 per partition)
OG = 8  # output blocks grouped per store DMA (8KB fp16 runs per partition)
X_SCALE = 2.0  # pre-scale before e3m4 quantization; 1/X_SCALE folded into W

E3 = ml_dtypes.float8_e3m4


def _build_program(R: int):
    nc = bacc.Bacc(
        "TRN2",
        target_bir_lowering=False,
        debug=False,
        enable_asserts=False,
        num_devices=N_CORES,
    )
    n_blocks = -(-R // BLOCK)
    n_pairs = -(-n_blocks // 2)

    xb = nc.dram_tensor(
        "xb", (n_pairs, 128, K_TILES, PAIR), mybir.dt.float8e3, kind="ExternalInput"
    ).ap()
    wb = nc.dram_tensor(
        "wb", (128, K_TILES, N_GENES), mybir.dt.float16, kind="ExternalInput"
    ).ap()
    bias = nc.dram_tensor(
        "bias", (N_GENES, 1), mybir.dt.float32, kind="ExternalInput"
    ).ap()
    outt = nc.dram_tensor(
        "outt", (N_GENES, R), mybir.dt.float16, kind="ExternalOutput"
    ).ap()

    # Store plan: blocks 0..n_big-1 accumulate in ONE SBUF-resident tile and
    # leave in a single store whose region dependency (every eviction in the
    # tile) fires only when block n_big-1 is evicted — which the PE reaches
    # just as the x stream drains.  So stores put ZERO traffic on the DMA
    # pool while x streams (the stream runs at the pool's knife edge, and
    # any mid-stream store burst used to stall the PE), yet the bulk store
    # still hides under the PE's last ~10us.  The trailing blocks store in
    # 2-block groups as soon as they're evicted, so the post-matmul drain is
    # only the final partial block.
    n_big = max(0, n_blocks - 5)
    sizes = []
    left = n_blocks - n_big
    while left > 0:
        g = min(2, left)
        sizes.append(g)
        left -= g
    group_of = {}
    j0 = n_big
    for g in sizes:
        group_of[j0] = g
        j0 += g

    with TileContext(nc) as tc:
        with (
            tc.tile_pool(name="const", bufs=1) as const_pool,
            tc.tile_pool(name="xp", bufs=8) as x_pool,
            tc.tile_pool(name="op", bufs=3) as out_pool,
            tc.tile_pool(name="ps", bufs=8, space="PSUM") as psum_pool,
        ):
            w_tile = const_pool.tile([128, K_TILES, N_GENES], mybir.dt.float16)
            nc.scalar.dma_start(out=w_tile[:], in_=wb[:])
            b_tile = const_pool.tile([N_GENES, 1], mybir.dt.float32)
            nc.scalar.dma_start(out=b_tile[:], in_=bias[:])
            o_big = None
            if n_big:
                o_big = const_pool.tile(
                    [N_GENES, n_big, BLOCK], mybir.dt.float16, tag="o_big"
                )

            gidx = -1
            for jp in range(n_pairs):
                # trailing rows beyond R are never loaded or computed
                pw = min(PAIR, R - jp * PAIR)
                x_tile = x_pool.tile([128, K_TILES, PAIR], mybir.dt.float8e3, tag="x")
                if jp == 0:
                    # split the first load along k so matmul 0 only waits for
                    # a quarter of the pair, shrinking the pipeline head
                    for kc in range(0, K_TILES, 2):
                        nc.sync.dma_start(
                            out=x_tile[:, kc : kc + 2, :pw],
                            in_=xb[jp, :, kc : kc + 2, :pw],
                        )
                else:
                    # each pair rides both HWDGE queues (half the k-tiles on
                    # each) so the pair's first half arrives in half the time
                    # and the k-ordered matmul chain can start on it
                    h = K_TILES // 2
                    nc.sync.dma_start(
                        out=x_tile[:, :h, :pw], in_=xb[jp, :, :h, :pw]
                    )
                    nc.scalar.dma_start(
                        out=x_tile[:, h:, :pw], in_=xb[jp, :, h:, :pw]
                    )
                for i in range(2):
                    j = 2 * jp + i
                    if j >= n_blocks:
                        break
                    bw = min(BLOCK, R - j * BLOCK)
                    psum = psum_pool.tile([N_GENES, BLOCK], mybir.dt.float32)
                    for k in range(K_TILES):
                        nc.tensor.matmul(
                            out=psum[:, :bw],
                            lhsT=w_tile[:, k, :],
                            rhs=x_tile[:, k, i * BLOCK : i * BLOCK + bw],
                            start=(k == 0),
                            stop=(k == K_TILES - 1),
                        )
                    if j < n_big:
                        nc.vector.tensor_scalar_add(
                            out=o_big[:, j, :bw],
                            in0=psum[:, :bw],
                            scalar1=b_tile[:],
                        )
                        if j == n_big - 1:
                            # bulk store: waits on every eviction above, so
                            # it fires right as the x stream drains
                            nc.sync.dma_start(
                                out=outt[:, : n_big * BLOCK],
                                in_=o_big.rearrange("p g r -> p (g r)")[:],
                            )
                        continue
                    if j in group_of:
                        gsize = group_of[j]
                        g0j = j
                        gidx += 1
                        o_tile = out_pool.tile(
                            [N_GENES, 2, BLOCK], mybir.dt.float16
                        )
                    nc.vector.tensor_scalar_add(
                        out=o_tile[:, j - g0j, :bw],
                        in0=psum[:, :bw],
                        scalar1=b_tile[:],
                    )
                    if j - g0j == gsize - 1:
                        g0 = g0j * BLOCK
                        gw = min(gsize * BLOCK, R - g0)
                        seng = nc.scalar if gidx % 2 == 0 else nc.sync
                        seng.dma_start(
                            out=outt[:, g0 : g0 + gw],
                            in_=o_tile.rearrange("p g r -> p (g r)")[:, :gw],
                        )

    nc.compile()
    return nc


def kernel(x, donor_labels, W, b):
    x = np.ascontiguousarray(x, dtype=np.float32)
    labels = np.asarray(donor_labels).astype(np.int64)
    W = np.asarray(W, dtype=np.float32)
    b = np.asarray(b, dtype=np.float32)
    B = x.shape[0]

    order = np.argsort(labels, kind="stable")
    counts = np.bincount(labels, minlength=N_DONORS)
    starts = np.zeros(N_DONORS + 1, dtype=np.int64)
    np.cumsum(counts, out=starts[1:])
    R = max(BLOCK, int(-(-counts.max() // 64)) * 64)
    n_blocks = -(-R // BLOCK)
    n_pairs = -(-n_blocks // 2)
    R_pad = n_pairs * PAIR

    in_maps = []
    idx_per_core = []
    for d in range(N_CORES):
        idx = order[starts[d] : starts[d + 1]]
        idx_per_core.append(idx)
        xr = np.zeros((R_pad, D_IN), dtype=E3)
        xr[: len(idx)] = (X_SCALE * x[idx]).astype(E3)
        # (jp*1024+r, k*128+p) -> (jp, p, k, r): one contiguous 8KB run per
        # partition per block-pair on the device side.
        xb = np.ascontiguousarray(
            xr.reshape(n_pairs, PAIR, K_TILES, 128).transpose(0, 3, 2, 1)
        )
        in_maps.append(
            {
                "xb": xb,
                "wb": np.ascontiguousarray(
                    (W[d].T / X_SCALE)
                    .reshape(K_TILES, 128, N_GENES)
                    .transpose(1, 0, 2)
                ).astype(np.float16),
                "bias": np.ascontiguousarray(b[d].reshape(N_GENES, 1)),
            }
        )

    nc = _build_program(R)

    try:
        res = run_bass_kernel_spmd(nc, in_maps, core_ids=list(range(N_CORES)))
    except Exception:
        # One retry: the axon-tunneled device occasionally drops a run with a
        # transient NRT exec error; a fresh dispatch succeeds.
        res = run_bass_kernel_spmd(nc, in_maps, core_ids=list(range(N_CORES)))

    out = np.empty((B, N_GENES), dtype=np.float32)
    for d in range(N_CORES):
        idx = idx_per_core[d]
        out[idx] = res.results[d]["outt"][:, : len(idx)].T
    return out


# revision 42
# speedup vs baseline: 1.1284x; 1.1284x over previous
"""Per-donor routed linear layer on 8 Trainium2 cores.

out[i] = x[i] @ W[donor_labels[i]].T + b[donor_labels[i]]

Strategy: route on host (stable sort of rows by donor label), one donor per
core, zero-padded to a common row count R.  Each core runs a dense
(R x 1024) @ (1024 x 100) matmul with K tiled 8x128, accumulating in PSUM,
bias added on the DVE during PSUM->SBUF eviction.

The kernel is memory-bound on x, so x is quantized host-side to fp8 e3m4
(float8e3, 4 mantissa bits) with a 2x pre-scale folded into W — the TRN2 PE
accepts mixed-dtype matmuls (stationary W in fp16, moving x in fp8e3, probed
exact to fp32 rounding), so W contributes no quantization error and the PE
still runs at 1 cycle/row.  Measured rel err vs the fp32 reference: 1.30e-2
(deterministic), against a 2e-2 gate.  fp8e4+DoubleRow (0.5 cyc/row) fails
the gate (x in e4m3 alone measures 2.55e-2), and the PE has no 8-bit int
mode, so 1 byte/element at 1 cyc/row is the floor for both traffic and PE.

With x at 1B the stream is balanced at the edge: per core ~16.9MB of x +
~3.3MB of fp16 output vs the 16-engine DMA pool's ~340GB/s, and 33*8
matmuls of 512 rows = ~58us on the PE — both ~58-60us.  Layout and queue
choices keep them overlapped:
- x is pre-permuted on host into (n_pairs, 128 partitions, 8 k-tiles, 1024
  rows) so each partition's read per block-pair is one contiguous 8KB run
  (descriptors are per partition-run; big runs keep the pool at line rate).
- each pair is issued half on the sync queue, half on the scalar queue (the
  only two HWDGE engines) so the k-ordered matmul chain starts on the first
  half while the second streams; the first pair is further split into 2-k-
  tile chunks because the DMA engine pool comes up staggered over ~10us.
- output (fp16, gene-major (100, R), transposed/unpermuted on host) is
  stored in 8-block groups on the gpsimd (SWDGE) queue to stay out of x's
  way, then 2-block groups on sync/scalar at the tail so the post-matmul
  drain is only the last 2 blocks.
Measured: ~77-80us NEFF exec (from 126us for the fp16 predecessor); PE-dense
window ~58us with <1us of gaps, ~11-13us head (DMA pool bring-up, engine
staggered ~650ns each), ~6us drain+teardown.
"""

import sys

sys.path.insert(0, "/opt/trn_rl_repo")

import numpy as np
import ml_dtypes

import concourse.bacc as bacc
import concourse.mybir as mybir
from concourse.tile import TileContext
from concourse.bass_utils import run_bass_kernel_spmd

N_CORES = 8
N_DONORS = 8
D_IN = 1024
N_GENES = 100
K_TILES = D_IN // 128
BLOCK = 512  # moving rows per matmul (one fp32 PSUM bank)
PAIR = 2 * BLOCK  # rows per x DMA (8KB fp8 per partition)
OG = 8  # output blocks grouped per store DMA (8KB fp16 runs per partition)
X_SCALE = 2.0  # pre-scale before e3m4 quantization; 1/X_SCALE folded into W

E3 = ml_dtypes.float8_e3m4


def _build_program(R: int):
    nc = bacc.Bacc(
        "TRN2",
        target_bir_lowering=False,
        debug=False,
        enable_asserts=False,
        num_devices=N_CORES,
    )
    n_blocks = -(-R // BLOCK)
    n_pairs = -(-n_blocks // 2)

    xb = nc.dram_tensor(
        "xb", (n_pairs, 128, K_TILES, PAIR), mybir.dt.float8e3, kind="ExternalInput"
    ).ap()
    wb = nc.dram_tensor(
        "wb", (128, K_TILES, N_GENES), mybir.dt.float16, kind="ExternalInput"
    ).ap()
    bias = nc.dram_tensor(
        "bias", (N_GENES, 1), mybir.dt.float32, kind="ExternalInput"
    ).ap()
    outt = nc.dram_tensor(
        "outt", (N_GENES, R), mybir.dt.float16, kind="ExternalOutput"
    ).ap()

    # out-store group sizes: OG blocks while x still streams (few, large
    # stores trickling between x pairs — the PE outlives x by only ~12us, so
    # store traffic MUST overlap the x phase; an all-deferred schedule was
    # measured 10us slower), then 2-block groups near the end so each store
    # issues as soon as its blocks are evicted and the post-matmul drain is
    # only the final 2 blocks
    sizes = []
    left = n_blocks
    while left > OG + 2:
        sizes.append(OG)
        left -= OG
    while left > 0:
        g = min(2, left)
        sizes.append(g)
        left -= g
    group_of = {}
    j0 = 0
    for g in sizes:
        group_of[j0] = g
        j0 += g

    with TileContext(nc) as tc:
        with (
            tc.tile_pool(name="const", bufs=1) as const_pool,
            tc.tile_pool(name="xp", bufs=8) as x_pool,
            tc.tile_pool(name="op", bufs=3) as out_pool,
            tc.tile_pool(name="ps", bufs=8, space="PSUM") as psum_pool,
        ):
            w_tile = const_pool.tile([128, K_TILES, N_GENES], mybir.dt.float16)
            nc.scalar.dma_start(out=w_tile[:], in_=wb[:])
            b_tile = const_pool.tile([N_GENES, 1], mybir.dt.float32)
            nc.scalar.dma_start(out=b_tile[:], in_=bias[:])
            gidx = -1
            for jp in range(n_pairs):
                # trailing rows beyond R are never loaded or computed
                pw = min(PAIR, R - jp * PAIR)
                x_tile = x_pool.tile([128, K_TILES, PAIR], mybir.dt.float8e3, tag="x")
                if jp == 0:
                    # split the first load along k so matmul 0 only waits for
                    # a quarter of the pair, shrinking the pipeline head
                    for kc in range(0, K_TILES, 2):
                        nc.sync.dma_start(
                            out=x_tile[:, kc : kc + 2, :pw],
                            in_=xb[jp, :, kc : kc + 2, :pw],
                        )
                else:
                    # each pair rides both HWDGE queues (half the k-tiles on
                    # each) so the pair's first half arrives in half the time
                    # and the k-ordered matmul chain can start on it
                    h = K_TILES // 2
                    nc.sync.dma_start(
                        out=x_tile[:, :h, :pw], in_=xb[jp, :, :h, :pw]
                    )
                    nc.scalar.dma_start(
                        out=x_tile[:, h:, :pw], in_=xb[jp, :, h:, :pw]
                    )
                for i in range(2):
                    j = 2 * jp + i
                    if j >= n_blocks:
                        break
                    bw = min(BLOCK, R - j * BLOCK)
                    psum = psum_pool.tile([N_GENES, BLOCK], mybir.dt.float32)
                    for k in range(K_TILES):
                        nc.tensor.matmul(
                            out=psum[:, :bw],
                            lhsT=w_tile[:, k, :],
                            rhs=x_tile[:, k, i * BLOCK : i * BLOCK + bw],
                            start=(k == 0),
                            stop=(k == K_TILES - 1),
                        )
                    if j in group_of:
                        gsize = group_of[j]
                        g0j = j
                        gidx += 1
                        o_tile = out_pool.tile(
                            [N_GENES, OG, BLOCK], mybir.dt.float16
                        )
                    nc.vector.tensor_scalar_add(
                        out=o_tile[:, j - g0j, :bw],
                        in0=psum[:, :bw],
                        scalar1=b_tile[:],
                    )
                    if j - g0j == gsize - 1:
                        g0 = g0j * BLOCK
                        gw = min(gsize * BLOCK, R - g0)
                        # the last few groups ride the HWDGE queues, whose x
                        # traffic has fully drained by then; earlier groups
                        # stay on gpsimd to keep out of x's way
                        if g0j >= n_blocks - 5:
                            seng = nc.sync if gidx % 2 == 0 else nc.scalar
                        else:
                            seng = nc.gpsimd
                        seng.dma_start(
                            out=outt[:, g0 : g0 + gw],
                            in_=o_tile.rearrange("p g r -> p (g r)")[:, :gw],
                        )

    nc.compile()
    return nc


def kernel(x, donor_labels, W, b):
    x = np.ascontiguousarray(x, dtype=np.float32)
    labels = np.asarray(donor_labels).astype(np.int64)
    W = np.asarray(W, dtype=np.float32)
    b = np.asarray(b, dtype=np.float32)
    B = x.shape[0]

    order = np.argsort(labels, kind="stable")
    counts = np.bincount(labels, minlength=N_DONORS)
    starts = np.zeros(N_DONORS + 1, dtype=np.int64)
    np.cumsum(counts, out=starts[1:])
    R = max(BLOCK, int(-(-counts.max() // 64)) * 64)
    n_blocks = -(-R // BLOCK)
    n_pairs = -(-n_blocks // 2)
    R_pad = n_pairs * PAIR

    in_maps = []
    idx_per_core = []
    for d in range(N_CORES):
        idx = order[starts[d] : starts[d + 1]]
        idx_per_core.append(idx)
        xr = np.zeros((R_pad, D_IN), dtype=E3)
        xr[: len(idx)] = (X_SCALE * x[idx]).astype(E3)
        # (jp*1024+r, k*128+p) -> (jp, p, k, r): one contiguous 8KB run per
        # partition per block-pair on the device side.
        xb = np.ascontiguousarray(
            xr.reshape(n_pairs, PAIR, K_TILES, 128).transpose(0, 3, 2, 1)
        )
        in_maps.append(
            {
                "xb": xb,
                "wb": np.ascontiguousarray(
                    (W[d].T / X_SCALE)
                    .reshape(K_TILES, 128, N_GENES)
                    .transpose(1, 0, 2)
                ).astype(np.float16),
                "bias": np.ascontiguousarray(b[d].reshape(N_GENES, 1)),
            }
        )

    nc = _build_program(R)

    try:
        res = run_bass_kernel_spmd(nc, in_maps, core_ids=list(range(N_CORES)))
    except Exception:
        # One retry: the axon-tunneled device occasionally drops a run with a
        # transient NRT exec error; a fresh dispatch succeeds.
        res = run_bass_kernel_spmd(nc, in_maps, core_ids=list(range(N_CORES)))

    out = np.empty((B, N_GENES), dtype=np.float32)
    for d in range(N_CORES):
        idx = idx_per_core[d]
        out[idx] = res.results[d]["outt"][:, : len(idx)].T
    return out
